# revision 36
# baseline (speedup 1.0000x reference)
"""Trainium2 Bass kernel for the SE-gated Non-local block (rank-1 attention).

Math (per batch item b, x viewed as [C, N] with N = H*W):
    S[c]    = sum_n x[c, n]                      (spatial sum)
    hid     = relu((se_w1 / N) @ S + se_b1)      (SE bottleneck; 1/N folds the mean)
    gate    = sigmoid(se_w2 @ hid + se_b2)       [C]
    w3e     = gate * [g_w | theta_w | phi_w]     [C, 3]   (gate folded into projections)
    proj    = w3e.T @ x                          [3, N]   (rows: g, theta, phi;
                                                 the 1x1-conv biases are zero)
    s_raw   = sum_n proj[0] * proj[2]
    out     = x + (A * s_raw) (outer) theta + Bc (outer) ones   where
              inv = bn_gamma / sqrt(bn_var + eps)
              A   = W_w * inv / N                (1/N folds the f/N normalizer)
              Bc  = (W_b - bn_mean) * inv + bn_beta

Memory-roofline design: the only mandatory HBM traffic is read-x + write-out
(37.7 MB/core, ~105 us at HBM rate).  x streams in as f32 over the sync
HWDGE ring into a 3-deep transient pool; ACT casts each chunk to a resident
bf16 copy (4.7 MB/item, so both items fit on-chip) and produces the spatial
sums for free on the cast's accumulate port.  Item 1 casts in half-chunks so
its SE gate is ready the moment its last load lands (its sums feed an
8-column SE matmul).  The store casts bf16->f32 inside the SWDGE DMA.

Precision: out = bf16(x) + correction, quantized to bf16 before the store
cast: ~2e-3 output rel err (bf16 mantissa), well inside the 2e-2 gate; the
correction term itself (rms ~5e-6 vs |x| ~ 1) runs in bf16 end-to-end.

Engine split: item 0's helper ops (psum copies, w3e) ride DVE, item 1's
ride ACT -- each the engine that is idle during that item's window (ACT is
busy casting item 1 during item 0's chain; DVE is busy with item 0's output
adds during item 1's chain).  The output affine (4x) + in-place add (2x)
are DVE for both.  Queues: sync ring = weight + x loads, scalar ring = the
g/phi row bounce, gpsimd SWDGE ring = the casting stores.

Sharding: pure data parallel, 2 of the 16 batch items per core, params
replicated, no collectives.
"""

import numpy as np

B, C, H, W = 16, 512, 96, 48
N = H * W            # 4608
HB = N // 2          # 2304 (item-1 half-chunk casts)
P = 128
KC = C // P          # 4 channel chunks
NB = 512             # free-dim block = one fp32 PSUM bank
NJ = N // NB         # 9
NCORES = 8
BPC = B // NCORES    # 2 batch items per core
SE_C = C // 16       # 32
BN_EPS = 1e-5

_CACHE = {}
LAST_RESULTS = None


def _build_bass():
    import concourse.mybir as mybir
    from concourse.bacc import Bacc
    from concourse.tile import TileContext

    f32 = mybir.dt.float32
    bf16 = mybir.dt.bfloat16
    AF = mybir.ActivationFunctionType
    AX = mybir.AxisListType
    ALU = mybir.AluOpType

    nc = Bacc()
    xs = nc.dram_tensor("xs", [BPC, C, N], f32, kind="ExternalInput")
    w1 = nc.dram_tensor("w1", [P, KC, SE_C], f32, kind="ExternalInput")
    w2 = nc.dram_tensor("w2", [SE_C, C], f32, kind="ExternalInput")
    b1 = nc.dram_tensor("b1", [SE_C, 1], f32, kind="ExternalInput")
    b2 = nc.dram_tensor("b2", [P, KC], f32, kind="ExternalInput")
    w3 = nc.dram_tensor("w3", [P, KC, 3], bf16, kind="ExternalInput")
    at = nc.dram_tensor("at", [P, KC], f32, kind="ExternalInput")   # A chunks
    bc = nc.dram_tensor("bc", [P, KC], f32, kind="ExternalInput")   # Bc chunks
    sel = nc.dram_tensor("sel", [3, P], bf16, kind="ExternalInput")  # theta row
    out_d = nc.dram_tensor("out", [BPC, C, N], f32, kind="ExternalOutput")
    tp_scr = nc.dram_tensor("tp_scr", [BPC, 3, N], bf16)

    MR = N // P  # 36: elements per partition in the reshaped g/phi rows

    with TileContext(nc) as tc:
        with (
            tc.tile_pool(name="wpool", bufs=1) as wpool,
            tc.tile_pool(name="xfpool", bufs=3) as xfpool,
            tc.tile_pool(name="xpool", bufs=2 * KC) as xpool,
            tc.tile_pool(name="ppool", bufs=2) as ppool,
            tc.tile_pool(name="spool", bufs=2) as spool,
            tc.tile_pool(name="tpool", bufs=2) as tpool,
            tc.tile_pool(name="ps_se", bufs=1, space="PSUM") as ps_se,
            tc.tile_pool(name="ps_pp", bufs=5, space="PSUM") as ps_pp,
            tc.tile_pool(name="ps_ub", bufs=2, space="PSUM") as ps_ub,
        ):
            w1t = wpool.tile([P, KC, SE_C], f32, tag="w1t")
            w2t = wpool.tile([SE_C, C], f32, tag="w2t")
            b1t = wpool.tile([SE_C, 1], f32, tag="b1t")
            b2t = wpool.tile([P, KC], f32, tag="b2t")
            w3t = wpool.tile([P, KC, 3], bf16, tag="w3t")
            att = wpool.tile([P, KC], f32, tag="att")
            bct = wpool.tile([P, KC], f32, tag="bct")
            selt = wpool.tile([3, P], bf16, tag="selt")
            on128 = wpool.tile([P, P], f32, tag="on128")  # all-ones (part. sum)
            warm = wpool.tile([P, 1], f32, tag="warm")

            nc.vector.memset(on128[:], 1.0)
            # pull the sigmoid ACT table load out of the critical chain
            nc.scalar.activation(out=warm[:], in_=on128[:, 0:1],
                                 func=AF.Sigmoid)
            # weights ride the scalar HWDGE ring so the x loads own the
            # sync ring from t=0 (8 small weight DMAs cost ~5us of ring)
            for t, d in ((w1t, w1), (w2t, w2), (b1t, b1), (b2t, b2),
                         (w3t, w3), (att, at), (bct, bc), (selt, sel)):
                nc.scalar.dma_start(out=t[:], in_=d[:])

            # ---- all 8 x-chunk loads enqueued upfront (plain f32 HWDGE;
            #      a casting SWDGE load runs ~20% below line rate) ----
            xfs = []
            for b in range(BPC):
                for k in range(KC):
                    xf = xfpool.tile([P, N], f32, tag="xf")
                    nc.sync.dma_start(out=xf[:], in_=xs[b, k * P:(k + 1) * P, :])
                    xfs.append(xf)

            # ---- ACT casts f32 -> resident bf16, spatial sums for free on
            #      the accumulate port.  Item 1 casts per half-chunk so its
            #      gate is ready right as its last load lands ----
            xbs = [[None] * KC for _ in range(BPC)]
            xp0 = spool.tile([P, KC], f32, tag="xp")
            xp1 = spool.tile([P, 2 * KC], f32, tag="xp")

            for k in range(KC):
                xb = xpool.tile([P, N], bf16, tag="xb")
                nc.scalar.activation(out=xb[:], in_=xfs[k][:],
                                     func=AF.Identity,
                                     accum_out=xp0[:, k:k + 1])
                xbs[0][k] = xb

            def sums1():
                for k in range(KC):
                    xb = xpool.tile([P, N], bf16, tag="xb")
                    for h in range(2):
                        sl = slice(h * HB, (h + 1) * HB)
                        nc.scalar.activation(
                            out=xb[:, sl], in_=xfs[KC + k][:, sl],
                            func=AF.Identity,
                            accum_out=xp1[:, 2 * k + h:2 * k + h + 1])
                    xbs[1][k] = xb

            def se_gate(xp, ncols):
                php = ps_se.tile([SE_C, 1], f32, tag="ps_se")
                for kk in range(ncols):
                    nc.tensor.matmul(php[:], w1t[:, kk * KC // ncols, :],
                                     xp[:, kk:kk + 1],
                                     start=(kk == 0), stop=(kk == ncols - 1))
                hid = spool.tile([SE_C, 1], f32, tag="hid")
                nc.scalar.activation(out=hid[:], in_=php[:], func=AF.Relu,
                                     bias=b1t[:], scale=1.0)
                gate = spool.tile([P, KC], f32, tag="gate")
                for k in range(KC):
                    gp = ps_se.tile([P, 1], f32, tag="ps_se")
                    nc.tensor.matmul(gp[:], w2t[:, k * P:(k + 1) * P], hid[:],
                                     start=True, stop=True)
                    nc.scalar.activation(out=gate[:, k:k + 1], in_=gp[:],
                                         func=AF.Sigmoid, bias=b2t[:, k:k + 1],
                                         scale=1.0)
                return gate

            def w3e_of(b, gate):
                # item 0 on DVE, item 1 on ACT: each the engine that's free
                # when that gate lands
                w3e = spool.tile([P, KC, 3], bf16, tag="w3e")
                for k in range(KC):
                    if b == 0:
                        nc.vector.tensor_scalar_mul(out=w3e[:, k, :],
                                                    in0=w3t[:, k, :],
                                                    scalar1=gate[:, k:k + 1])
                    else:
                        nc.scalar.activation(out=w3e[:, k, :],
                                             in_=w3t[:, k, :],
                                             func=AF.Identity,
                                             scale=gate[:, k:k + 1])
                return w3e

            def copy_psum(b, out, in_):
                # item 0's psum->SBUF copies on DVE, item 1's on ACT
                if b == 0:
                    nc.vector.tensor_copy(out=out, in_=in_)
                else:
                    nc.scalar.activation(out=out, in_=in_, func=AF.Identity,
                                         scale=1.0)

            def proj_phase(b, w3e):
                # gated projections proj = w3e.T @ x (bf16 PE) + theta
                # broadcast to all 128 partitions (selector matmul).
                # k-outer within PSUM-bank groups: consecutive matmuls keep
                # the same stationary weights loaded, so the PE streams at
                # ~216ns/block instead of stalling ~430ns on every weight
                # swap (walrus runs with ldweights-dedup disabled).
                proj = ppool.tile([3, N], bf16, tag="proj")
                ubt = tpool.tile([P, N], bf16, tag="ubt")
                for grp in (range(0, 5), range(5, NJ)):
                    pps = {}
                    for j in grp:
                        pps[j] = ps_pp.tile([3, NB], f32, tag="pp",
                                            name=f"pp{j}")
                    for k in range(KC):
                        for j in grp:
                            sl = slice(j * NB, (j + 1) * NB)
                            nc.tensor.matmul(pps[j][:], w3e[:, k, :],
                                             xbs[b][k][:, sl],
                                             start=(k == 0),
                                             stop=(k == KC - 1))
                    for j in grp:
                        sl = slice(j * NB, (j + 1) * NB)
                        copy_psum(b, proj[:, sl], pps[j][:])
                for grp in (range(0, 5), range(5, NJ)):
                    for j in grp:
                        sl = slice(j * NB, (j + 1) * NB)
                        ub_ps = ps_ub.tile([P, NB], f32, tag="ub_ps")
                        nc.tensor.matmul(ub_ps[:], selt[:], proj[:, sl],
                                         start=True, stop=True)
                        copy_psum(b, ubt[:, sl], ub_ps[:])
                return proj, ubt

            def dot_phase(b, proj):
                # g,phi -> [128, 2, 36] via a DRAM bounce on the scalar
                # HWDGE ring (one combined readback) so the dot product
                # uses every DVE lane; cross-partition sum via ones-matmul
                gp_rs = spool.tile([P, 2, MR], bf16, tag="gp_rs")
                nc.scalar.dma_start(out=tp_scr[b], in_=proj[:])
                nc.scalar.dma_start(
                    out=gp_rs[:],
                    in_=tp_scr[b, 0:3:2, :].rearrange("r (p m) -> p r m",
                                                      p=P))
                prod = spool.tile([P, MR], f32, tag="prod")
                nc.vector.tensor_mul(out=prod[:], in0=gp_rs[:, 0, :],
                                     in1=gp_rs[:, 1, :])
                r1 = spool.tile([P, 1], f32, tag="r1")
                nc.vector.reduce_sum(out=r1[:], in_=prod[:], axis=AX.X)
                sb = ps_se.tile([P, 1], f32, tag="ps_se")
                nc.tensor.matmul(sb[:], on128[:], r1[:], start=True, stop=True)
                ast = spool.tile([P, KC], f32, tag="ast")
                nc.vector.tensor_scalar_mul(out=ast[:], in0=att[:],
                                            scalar1=sb[:])
                return ast

            def out_chunk(b, k, ubt, ast):
                # out = bf16(x) + (A*s)*theta + Bc in place; store casts
                # bf16->f32 in the DMA.  Affine 4x + add 2x, both DVE.
                t1 = tpool.tile([P, N], bf16, tag="t1")
                nc.vector.tensor_scalar(out=t1[:], in0=ubt[:],
                                        scalar1=ast[:, k:k + 1],
                                        scalar2=bct[:, k:k + 1],
                                        op0=ALU.mult, op1=ALU.add)
                nc.vector.tensor_add(out=xbs[b][k][:],
                                     in0=xbs[b][k][:], in1=t1[:])
                nc.gpsimd.dma_start(out=out_d[b, k * P:(k + 1) * P, :],
                                    in_=xbs[b][k][:])

            gate0 = se_gate(xp0, KC)
            w3e0 = w3e_of(0, gate0)
            proj0, ubt0 = proj_phase(0, w3e0)
            ast0 = dot_phase(0, proj0)
            # Logical-timestamp floor: the static scheduler orders the ACT
            # queue by its own optimistic ready-time estimates and would
            # otherwise slot item 1's casts ahead of item 0's relu/sigmoid,
            # stalling item 0's whole chain.  40us is past any estimate of
            # item 0's SE but at/below when item 1's loads actually land.
            with tc.tile_wait_until(0.04):
                sums1()
            gate1 = se_gate(xp1, 2 * KC)
            w3e1 = w3e_of(1, gate1)
            for k in range(KC):
                out_chunk(0, k, ubt0, ast0)
            proj1, ubt1 = proj_phase(1, w3e1)
            ast1 = dot_phase(1, proj1)
            for k in range(KC):
                out_chunk(1, k, ubt1, ast1)

    nc.finalize()  # runs Bacc compile passes (wait splitting, reg alloc, ...)
    return nc


def kernel(**inputs):
    global LAST_RESULTS
    from concourse.bass_utils import run_bass_kernel_spmd

    a = {k: np.asarray(v, dtype=np.float32) for k, v in inputs.items()}
    x = np.ascontiguousarray(a["x"]).reshape(B, C, N)

    inv = a["bn_gamma"] / np.sqrt(a["bn_var"] + BN_EPS)
    A = (a["W_w"] * inv / N).astype(np.float32)
    Bc = ((a["W_b"] - a["bn_mean"]) * inv + a["bn_beta"]).astype(np.float32)

    # the kernel folds the (always-zero) g/theta/phi conv biases away
    assert abs(float(a["g_b"])) < 1e-30 and abs(float(a["theta_b"])) < 1e-30 \
        and abs(float(a["phi_b"])) < 1e-30, "nonzero projection bias"

    w1h = np.ascontiguousarray(
        (a["se_w1"] / N).T.reshape(KC, P, SE_C).transpose(1, 0, 2)).astype(np.float32)
    w2h = np.ascontiguousarray(a["se_w2"].T).astype(np.float32)
    b1h = np.ascontiguousarray(a["se_b1"].reshape(SE_C, 1))
    b2h = np.ascontiguousarray(a["se_b2"].reshape(KC, P).T)
    import ml_dtypes
    w3h = np.ascontiguousarray(
        np.stack([a["g_w"], a["theta_w"], a["phi_w"]], axis=1)
        .reshape(KC, P, 3).transpose(1, 0, 2)).astype(ml_dtypes.bfloat16)
    ath = np.ascontiguousarray(A.reshape(KC, P).T)
    bch = np.ascontiguousarray(Bc.reshape(KC, P).T)
    selh = np.zeros((3, P), dtype=ml_dtypes.bfloat16)
    selh[1, :] = 1.0

    if "nc" not in _CACHE:
        _CACHE["nc"] = _build_bass()
    nc = _CACHE["nc"]

    in_maps = []
    for c in range(NCORES):
        in_maps.append({
            "xs": np.ascontiguousarray(x[c * BPC:(c + 1) * BPC]),
            "w1": w1h, "w2": w2h, "b1": b1h, "b2": b2h,
            "w3": w3h, "at": ath, "bc": bch, "sel": selh,
        })

    res = run_bass_kernel_spmd(nc, in_maps, core_ids=list(range(NCORES)))
    LAST_RESULTS = res

    out = np.concatenate([res.results[c]["out"] for c in range(NCORES)], axis=0)
    return np.ascontiguousarray(out.reshape(B, C, H, W))
